# revision 1
# baseline (speedup 1.0000x reference)
"""Trainium2 Bass kernel for the RNN-T style Joint network:

    out[b,t,u,v] = sum_k tanh(enc_p[b,t,k] + dec_p[b,u,k] + b1[k]) * W2[v,k] + b2[v]
    enc_p = h_enc @ W1[:, :H].T ; dec_p = h_dec @ W1[:, H:].T

Sharding: data-parallel over B across 8 NeuronCores (B == 8, one batch row per
core). Weights are replicated. No collectives needed.

Per-core pipeline (all on one NeuronCore):
  GEMM1 (fp32):  enc_p [T,HID], dec_p+b1 [U,HID] via PE, evacuated to bf16.
  hT build (PE): for each 512-wide TU chunk, psum[j, t'*64+u] =
                 encb[t,j] (via selector matmul) + decb[u,j] (via tiled-identity
                 matmul) accumulated in fp32 PSUM. This materializes the
                 broadcast-add entirely on the TensorEngine.
  tanh (ScalarE): PSUM -> SBUF bf16, giving hT [HID, TU-chunk] (transposed
                 layout = ready to be the stationary operand of GEMM2).
  GEMM2 (PE, bf16): out[tu, v] = hT.T @ W2T accumulated over 5 K-tiles in
                 fp32 PSUM.
  b2 add (VectorE): PSUM + b2 -> SBUF fp32 out tile [128, 1024].
  DMA out: contiguous 512KB stores.
"""

import numpy as np
import ml_dtypes

B, T, U, H = 8, 256, 64, 512
HID, V = 640, 1024
TU = T * U  # 16384
N_CORES = 8
N_CHUNKS = TU // 512  # 32 chunks of 8 t-values x 64 u-values

BF16 = ml_dtypes.bfloat16

_CACHE = {}


def _build_consts():
    # sel[s][k, t'*64 + u] = 1 iff k == s*8 + t'   (k: t-index within the
    # 128-row t-tile; each chunk covers 8 t values x 64 u values)
    sel = np.zeros((16, 128, 512), dtype=BF16)
    for s in range(16):
        for tp in range(8):
            sel[s, s * 8 + tp, tp * 64:(tp + 1) * 64] = 1.0
    # i64t[u', t'*64 + u] = 1 iff u' == u
    i64t = np.zeros((64, 512), dtype=BF16)
    eye = np.eye(64, dtype=BF16)
    for tp in range(8):
        i64t[:, tp * 64:(tp + 1) * 64] = eye
    return sel.reshape(16 * 128, 512), i64t


def _build_bass():
    import concourse.tile as tile
    from concourse import bacc, mybir

    f32 = mybir.dt.float32
    bf16 = mybir.dt.bfloat16
    Tanh = mybir.ActivationFunctionType.Tanh

    nc = bacc.Bacc("TRN2", target_bir_lowering=False, debug=False,
                   num_devices=N_CORES)

    hencT = nc.dram_tensor("hencT", [H, T], f32, kind="ExternalInput").ap()
    hdecT = nc.dram_tensor("hdecT", [H, U], f32, kind="ExternalInput").ap()
    w1T = nc.dram_tensor("w1T", [2 * H, HID], f32, kind="ExternalInput").ap()
    w2T = nc.dram_tensor("w2T", [HID, V], bf16, kind="ExternalInput").ap()
    b1row = nc.dram_tensor("b1row", [1, HID], f32, kind="ExternalInput").ap()
    b2rep = nc.dram_tensor("b2rep", [128, V], f32, kind="ExternalInput").ap()
    sel = nc.dram_tensor("sel", [16 * 128, 512], bf16, kind="ExternalInput").ap()
    i64t = nc.dram_tensor("i64t", [U, 512], bf16, kind="ExternalInput").ap()
    out = nc.dram_tensor("out", [TU, V], f32, kind="ExternalOutput").ap()

    with tile.TileContext(nc) as tc:
        with (
            tc.tile_pool(name="consts", bufs=1) as consts,
            tc.tile_pool(name="psum", bufs=1, space="PSUM") as psum,
            tc.tile_pool(name="hTp", bufs=2) as hTp,
            tc.tile_pool(name="outp", bufs=4) as outp,
        ):
            # ---- load constants / inputs into SBUF ----
            henc_t = []
            for k in range(4):
                t_ = consts.tile([128, T], f32, tag=f"hencT{k}", name=f"hencT{k}")
                nc.sync.dma_start(out=t_, in_=hencT[k * 128:(k + 1) * 128, :])
                henc_t.append(t_)
            hdec_t = []
            for k in range(4):
                t_ = consts.tile([128, U], f32, tag=f"hdecT{k}", name=f"hdecT{k}")
                nc.sync.dma_start(out=t_, in_=hdecT[k * 128:(k + 1) * 128, :])
                hdec_t.append(t_)
            w1_t = []
            for k in range(8):
                t_ = consts.tile([128, HID], f32, tag=f"w1T{k}", name=f"w1T{k}")
                nc.sync.dma_start(out=t_, in_=w1T[k * 128:(k + 1) * 128, :])
                w1_t.append(t_)
            w2_t = []
            for k in range(5):
                t_ = consts.tile([128, V], bf16, tag=f"w2T{k}", name=f"w2T{k}")
                nc.sync.dma_start(out=t_, in_=w2T[k * 128:(k + 1) * 128, :])
                w2_t.append(t_)
            sel_t = []
            for s in range(16):
                t_ = consts.tile([128, 512], bf16, tag=f"sel{s}", name=f"sel{s}")
                nc.sync.dma_start(out=t_, in_=sel[s * 128:(s + 1) * 128, :])
                sel_t.append(t_)
            i64_t = consts.tile([64, 512], bf16, tag="i64", name="i64")
            nc.sync.dma_start(out=i64_t, in_=i64t[:, :])
            b1_t = consts.tile([1, HID], f32, tag="b1", name="b1")
            nc.sync.dma_start(out=b1_t, in_=b1row[:, :])
            b2_t = consts.tile([128, V], f32, tag="b2", name="b2")
            nc.sync.dma_start(out=b2_t, in_=b2rep[:, :])
            ones_t = consts.tile([1, U], f32, tag="ones", name="ones")
            nc.vector.memset(ones_t, 1.0)

            encb = []
            for tt in range(2):
                t_ = consts.tile([128, HID], bf16, tag=f"encb{tt}", name=f"encb{tt}")
                encb.append(t_)
            decb = consts.tile([64, HID], bf16, tag="decb", name="decb")

            # ---- GEMM1: enc_p [T, HID] (fp32), evacuate to bf16 ----
            col_blocks = [(0, 512), (512, 128)]
            for tt in range(2):
                for c0, cw in col_blocks:
                    ps = psum.tile([128, 512], f32, tag="g1", bufs=1,
                                   name=f"ps1e{tt}{c0}")
                    for k in range(4):
                        nc.tensor.matmul(
                            ps[:, :cw],
                            lhsT=henc_t[k][:, tt * 128:(tt + 1) * 128],
                            rhs=w1_t[k][:, c0:c0 + cw],
                            start=(k == 0), stop=(k == 3),
                        )
                    nc.scalar.copy(out=encb[tt][:, c0:c0 + cw], in_=ps[:, :cw])

            # ---- GEMM1: dec_p + b1 [U, HID] (fp32), evacuate to bf16 ----
            for c0, cw in col_blocks:
                ps = psum.tile([64, 512], f32, tag="g1", bufs=1, name=f"ps1d{c0}")
                for k in range(4):
                    nc.tensor.matmul(
                        ps[:, :cw],
                        lhsT=hdec_t[k][:, 0:U],
                        rhs=w1_t[4 + k][:, c0:c0 + cw],
                        start=(k == 0), stop=False,
                    )
                nc.tensor.matmul(
                    ps[:, :cw],
                    lhsT=ones_t[0:1, 0:U],
                    rhs=b1_t[0:1, c0:c0 + cw],
                    start=False, stop=True,
                )
                nc.scalar.copy(out=decb[:, c0:c0 + cw], in_=ps[:, :cw])

            # ---- main loop over 32 TU chunks of 512 ----
            for c in range(N_CHUNKS):
                tt, s = c // 16, c % 16
                hts = []
                for kk in range(5):
                    ps = psum.tile([128, 512], f32, tag="build", bufs=4,
                                   name=f"psb{c}_{kk}")
                    nc.tensor.matmul(
                        ps, lhsT=encb[tt][:, kk * 128:(kk + 1) * 128],
                        rhs=sel_t[s], start=True, stop=False,
                    )
                    nc.tensor.matmul(
                        ps, lhsT=decb[:, kk * 128:(kk + 1) * 128],
                        rhs=i64_t, start=False, stop=True,
                    )
                    ht = hTp.tile([128, 512], bf16, tag=f"hT{kk}",
                                  name=f"hT{c}_{kk}")
                    nc.scalar.activation(out=ht, in_=ps, func=Tanh)
                    hts.append(ht)

                for mt in range(4):
                    ot = outp.tile([128, V], f32, tag="out", name=f"out{c}_{mt}")
                    for vc in range(2):
                        ps2 = psum.tile([128, 512], f32, tag="g2", bufs=3,
                                        name=f"ps2_{c}_{mt}_{vc}")
                        for kk in range(5):
                            nc.tensor.matmul(
                                ps2,
                                lhsT=hts[kk][:, mt * 128:(mt + 1) * 128],
                                rhs=w2_t[kk][:, vc * 512:(vc + 1) * 512],
                                start=(kk == 0), stop=(kk == 4),
                            )
                        nc.vector.tensor_add(
                            out=ot[:, vc * 512:(vc + 1) * 512],
                            in0=ps2,
                            in1=b2_t[:, vc * 512:(vc + 1) * 512],
                        )
                    r0 = c * 512 + mt * 128
                    nc.sync.dma_start(out=out[r0:r0 + 128, :], in_=ot)

    nc.finalize()
    return nc


def _get_nc():
    if "nc" not in _CACHE:
        _CACHE["nc"] = _build_bass()
    return _CACHE["nc"]


def _make_in_maps(h_enc, h_dec, W1, b1, W2, b2):
    h_enc = np.asarray(h_enc, dtype=np.float32)
    h_dec = np.asarray(h_dec, dtype=np.float32)
    W1 = np.asarray(W1, dtype=np.float32)
    b1 = np.asarray(b1, dtype=np.float32)
    W2 = np.asarray(W2, dtype=np.float32)
    b2 = np.asarray(b2, dtype=np.float32)

    w1T = np.ascontiguousarray(W1.T)                    # [2H, HID] f32
    w2T = np.ascontiguousarray(W2.T).astype(BF16)       # [HID, V] bf16
    b1row = np.ascontiguousarray(b1.reshape(1, HID))
    b2rep = np.ascontiguousarray(np.tile(b2.reshape(1, V), (128, 1)))
    sel, i64t = _build_consts()

    in_maps = []
    for b in range(N_CORES):
        in_maps.append({
            "hencT": np.ascontiguousarray(h_enc[b].T),  # [H, T]
            "hdecT": np.ascontiguousarray(h_dec[b].T),  # [H, U]
            "w1T": w1T,
            "w2T": w2T,
            "b1row": b1row,
            "b2rep": b2rep,
            "sel": sel,
            "i64t": i64t,
        })
    return in_maps


def _run(in_maps, **kwargs):
    from concourse import bass_utils
    nc = _get_nc()
    return bass_utils.run_bass_kernel_spmd(
        nc, in_maps, core_ids=list(range(N_CORES)), **kwargs)


def kernel(h_enc, h_dec, W1, b1, W2, b2):
    in_maps = _make_in_maps(h_enc, h_dec, W1, b1, W2, b2)
    res = _run(in_maps)
    outs = [r["out"].reshape(T, U, V) for r in res.results]
    return np.stack(outs, axis=0)


# revision 2
# speedup vs baseline: 54054.2322x; 54054.2322x over previous
"""Trainium2 Bass kernel for the RNN-T style Joint network:

    out[b,t,u,v] = sum_k tanh(enc_p[b,t,k] + dec_p[b,u,k] + b1[k]) * W2[v,k] + b2[v]
    enc_p = h_enc @ W1[:, :H].T ; dec_p = h_dec @ W1[:, H:].T

Sharding: data-parallel over B across 8 NeuronCores (B == 8, one batch row per
core). Weights are replicated. No collectives needed.

Per-core pipeline (one NeuronCore):
  GEMM1 (fp32, PE): enc_pT [HID, T] and dec_pT [HID, U] computed directly in
      transposed layout (HID on partitions); b1 folded in via the ScalarE
      per-partition activation bias during PSUM->SBUF evacuation.
  broadcast-add (VectorE): pre[j, t'*64+u] = encbT[j, t] + decT[j, u] in ONE
      tensor_add per [128, 512] block using stride-0 broadcast access
      patterns (verified supported by the DVE).
  tanh (ScalarE): SBUF fp32 -> SBUF bf16, producing hT [HID, TU-chunk] --
      already transposed to be the stationary operand of GEMM2.
  GEMM2 (PE, bf16): out[tu, v] = hT.T @ W2T accumulated over 5 K-tiles in
      fp32 PSUM (1280 N=512 matmuls: the roofline term).
  b2 add (VectorE): PSUM + b2rep -> SBUF fp32 out tile [128, 1024].
  DMA out: contiguous 512KB stores.

The build for chunk c+2 is emitted before GEMM2 of chunk c so the in-order
VectorE queue always runs the next chunk's broadcast-adds ahead of the
current chunk's evacuations, keeping the PE from stalling on hT tiles.
"""

import numpy as np
import ml_dtypes

B, T, U, H = 8, 256, 64, 512
HID, V = 640, 1024
TU = T * U  # 16384
N_CORES = 8
N_CHUNKS = TU // 512  # 32 chunks of 8 t-values x 64 u-values
KK = HID // 128  # 5 K-tiles

BF16 = ml_dtypes.bfloat16

_CACHE = {}


def _build_bass():
    import concourse.bass as bass
    import concourse.tile as tile
    from concourse import bacc, mybir

    f32 = mybir.dt.float32
    bf16 = mybir.dt.bfloat16
    Tanh = mybir.ActivationFunctionType.Tanh

    nc = bacc.Bacc("TRN2", target_bir_lowering=False, debug=False,
                   num_devices=N_CORES)

    hencT = nc.dram_tensor("hencT", [H, T], f32, kind="ExternalInput").ap()
    hdecT = nc.dram_tensor("hdecT", [H, U], f32, kind="ExternalInput").ap()
    w1T = nc.dram_tensor("w1T", [2 * H, HID], f32, kind="ExternalInput").ap()
    w2T = nc.dram_tensor("w2T", [HID, V], bf16, kind="ExternalInput").ap()
    b1col = nc.dram_tensor("b1col", [HID, 1], f32, kind="ExternalInput").ap()
    b2rep = nc.dram_tensor("b2rep", [128, V], f32, kind="ExternalInput").ap()
    out = nc.dram_tensor("out", [TU, V], f32, kind="ExternalOutput").ap()

    def bcast3(ap2d, mid):
        """[P, N] AP -> [P, mid, N] with a stride-0 middle dim."""
        return bass.AP(tensor=ap2d.tensor, offset=ap2d.offset,
                       ap=[ap2d.ap[0], [0, mid], ap2d.ap[1]])

    def repeat3(ap2d, inner):
        """[P, N] AP -> [P, N, inner] with a stride-0 inner dim."""
        return bass.AP(tensor=ap2d.tensor, offset=ap2d.offset,
                       ap=[ap2d.ap[0], ap2d.ap[1], [0, inner]])

    with tile.TileContext(nc) as tc:
        with (
            tc.tile_pool(name="consts", bufs=1) as consts,
            tc.tile_pool(name="psum", bufs=1, space="PSUM") as psum,
            tc.tile_pool(name="prep", bufs=4) as prep,
            tc.tile_pool(name="hTp", bufs=3) as hTp,
            tc.tile_pool(name="outp", bufs=4) as outp,
        ):
            # ---- load inputs into SBUF ----
            henc_t = []
            for k in range(4):
                t_ = consts.tile([128, T], f32, tag=f"hencT{k}", name=f"hencT{k}")
                nc.sync.dma_start(out=t_, in_=hencT[k * 128:(k + 1) * 128, :])
                henc_t.append(t_)
            hdec_t = []
            for k in range(4):
                t_ = consts.tile([128, U], f32, tag=f"hdecT{k}", name=f"hdecT{k}")
                nc.sync.dma_start(out=t_, in_=hdecT[k * 128:(k + 1) * 128, :])
                hdec_t.append(t_)
            w1_t = []
            for k in range(8):
                t_ = consts.tile([128, HID], f32, tag=f"w1T{k}", name=f"w1T{k}")
                nc.sync.dma_start(out=t_, in_=w1T[k * 128:(k + 1) * 128, :])
                w1_t.append(t_)
            w2_t = []
            for k in range(KK):
                t_ = consts.tile([128, V], bf16, tag=f"w2T{k}", name=f"w2T{k}")
                nc.sync.dma_start(out=t_, in_=w2T[k * 128:(k + 1) * 128, :])
                w2_t.append(t_)
            b1_t = []
            for kk in range(KK):
                t_ = consts.tile([128, 1], f32, tag=f"b1{kk}", name=f"b1{kk}")
                nc.sync.dma_start(out=t_, in_=b1col[kk * 128:(kk + 1) * 128, :])
                b1_t.append(t_)
            b2_t = consts.tile([128, V], f32, tag="b2", name="b2")
            nc.sync.dma_start(out=b2_t, in_=b2rep[:, :])

            # ---- GEMM1 (fp32): enc_pT [HID, T], dec_pT [HID, U] ----
            encbT = []
            for kk in range(KK):
                ps = psum.tile([128, T], f32, tag="g1", bufs=2, name=f"pse{kk}")
                for k in range(4):
                    nc.tensor.matmul(
                        ps,
                        lhsT=w1_t[k][:, kk * 128:(kk + 1) * 128],
                        rhs=henc_t[k],
                        start=(k == 0), stop=(k == 3),
                    )
                e_ = consts.tile([128, T], f32, tag=f"encbT{kk}", name=f"encbT{kk}")
                # encbT = enc_pT + b1 (per-partition bias)
                nc.scalar.add(out=e_, in_=ps, add=b1_t[kk])
                encbT.append(e_)
            decT = []
            for kk in range(KK):
                ps = psum.tile([128, U], f32, tag="g1", bufs=2, name=f"psd{kk}")
                for k in range(4):
                    nc.tensor.matmul(
                        ps,
                        lhsT=w1_t[4 + k][:, kk * 128:(kk + 1) * 128],
                        rhs=hdec_t[k],
                        start=(k == 0), stop=(k == 3),
                    )
                d_ = consts.tile([128, U], f32, tag=f"decT{kk}", name=f"decT{kk}")
                nc.scalar.copy(out=d_, in_=ps)
                decT.append(d_)

            # ---- main loop: build is emitted 2 chunks ahead of GEMM2 ----
            hT_by_chunk = {}

            def emit_build(c):
                hts = []
                for kk in range(KK):
                    pre = prep.tile([128, 512], f32, tag=f"pre{kk}",
                                    name=f"pre{c}_{kk}")
                    pre_ap = pre[:, :]
                    out3 = bass.AP(tensor=pre_ap.tensor, offset=pre_ap.offset,
                                   ap=[pre_ap.ap[0], [64, 8], [1, 64]])
                    nc.vector.tensor_add(
                        out=out3,
                        in0=bcast3(decT[kk][:, :], 8),
                        in1=repeat3(encbT[kk][:, c * 8:(c + 1) * 8], 64),
                    )
                    ht = hTp.tile([128, 512], bf16, tag=f"hT{kk}",
                                  name=f"hT{c}_{kk}")
                    nc.scalar.activation(out=ht, in_=pre, func=Tanh)
                    hts.append(ht)
                hT_by_chunk[c] = hts

            emit_build(0)
            emit_build(1)
            for c in range(N_CHUNKS):
                if c + 2 < N_CHUNKS:
                    emit_build(c + 2)
                hts = hT_by_chunk.pop(c)
                for mt in range(4):
                    ot = outp.tile([128, V], f32, tag="out", name=f"out{c}_{mt}")
                    for vc in range(2):
                        ps2 = psum.tile([128, 512], f32, tag="g2", bufs=6,
                                        name=f"ps2_{c}_{mt}_{vc}")
                        for kk in range(KK):
                            nc.tensor.matmul(
                                ps2,
                                lhsT=hts[kk][:, mt * 128:(mt + 1) * 128],
                                rhs=w2_t[kk][:, vc * 512:(vc + 1) * 512],
                                start=(kk == 0), stop=(kk == KK - 1),
                            )
                        nc.vector.tensor_add(
                            out=ot[:, vc * 512:(vc + 1) * 512],
                            in0=ps2,
                            in1=b2_t[:, vc * 512:(vc + 1) * 512],
                        )
                    r0 = c * 512 + mt * 128
                    nc.sync.dma_start(out=out[r0:r0 + 128, :], in_=ot)

    nc.finalize()
    return nc


def _get_nc():
    if "nc" not in _CACHE:
        _CACHE["nc"] = _build_bass()
    return _CACHE["nc"]


def _make_in_maps(h_enc, h_dec, W1, b1, W2, b2):
    h_enc = np.asarray(h_enc, dtype=np.float32)
    h_dec = np.asarray(h_dec, dtype=np.float32)
    W1 = np.asarray(W1, dtype=np.float32)
    b1 = np.asarray(b1, dtype=np.float32)
    W2 = np.asarray(W2, dtype=np.float32)
    b2 = np.asarray(b2, dtype=np.float32)

    w1T = np.ascontiguousarray(W1.T)                    # [2H, HID] f32
    w2T = np.ascontiguousarray(W2.T).astype(BF16)       # [HID, V] bf16
    b1col = np.ascontiguousarray(b1.reshape(HID, 1))
    b2rep = np.ascontiguousarray(np.tile(b2.reshape(1, V), (128, 1)))

    in_maps = []
    for b in range(N_CORES):
        in_maps.append({
            "hencT": np.ascontiguousarray(h_enc[b].T),  # [H, T]
            "hdecT": np.ascontiguousarray(h_dec[b].T),  # [H, U]
            "w1T": w1T,
            "w2T": w2T,
            "b1col": b1col,
            "b2rep": b2rep,
        })
    return in_maps


def _run(in_maps, **kwargs):
    from concourse import bass_utils
    nc = _get_nc()
    return bass_utils.run_bass_kernel_spmd(
        nc, in_maps, core_ids=list(range(N_CORES)), **kwargs)


def kernel(h_enc, h_dec, W1, b1, W2, b2):
    in_maps = _make_in_maps(h_enc, h_dec, W1, b1, W2, b2)
    res = _run(in_maps)
    outs = [r["out"].reshape(T, U, V) for r in res.results]
    return np.stack(outs, axis=0)


# revision 3
# speedup vs baseline: 54357.0033x; 1.0056x over previous
"""Trainium2 Bass kernel for the RNN-T style Joint network:

    out[b,t,u,v] = sum_k tanh(enc_p[b,t,k] + dec_p[b,u,k] + b1[k]) * W2[v,k] + b2[v]
    enc_p = h_enc @ W1[:, :H].T ; dec_p = h_dec @ W1[:, H:].T

Sharding: data-parallel over B across 8 NeuronCores (B == 8, one batch row per
core). Weights are replicated. No collectives needed.

Per-core pipeline (one NeuronCore):
  GEMM1 (fp32, PE): enc_pT [HID, T] and dec_pT [HID, U] computed directly in
      transposed layout (HID on partitions); b1 folded in via the ScalarE
      per-partition activation bias during PSUM->SBUF evacuation.
  broadcast-add (VectorE): pre[j, t'*64+u] = encbT[j, t] + decT[j, u] in ONE
      tensor_add per [128, 512] block using stride-0 broadcast access
      patterns (verified supported by the DVE).
  tanh (ScalarE): SBUF fp32 -> SBUF bf16, producing hT [HID, TU-chunk] --
      already transposed to be the stationary operand of GEMM2.
  GEMM2 (PE, bf16): out[tu, v] = hT.T @ W2T accumulated over 5 K-tiles in
      fp32 PSUM (1280 N=512 matmuls: the roofline term).
  b2 add (VectorE): PSUM + b2rep -> SBUF fp32 out tile [128, 1024].
  DMA out: contiguous 512KB stores.

The build for chunk c+2 is emitted before GEMM2 of chunk c so the in-order
VectorE queue always runs the next chunk's broadcast-adds ahead of the
current chunk's evacuations, keeping the PE from stalling on hT tiles.
"""

import numpy as np
import ml_dtypes

B, T, U, H = 8, 256, 64, 512
HID, V = 640, 1024
TU = T * U  # 16384
N_CORES = 8
N_CHUNKS = TU // 1024  # 16 chunks of 16 t-values x 64 u-values
KK = HID // 128  # 5 K-tiles

BF16 = ml_dtypes.bfloat16

_CACHE = {}


def _build_bass():
    import concourse.bass as bass
    import concourse.tile as tile
    from concourse import bacc, mybir

    f32 = mybir.dt.float32
    bf16 = mybir.dt.bfloat16
    Tanh = mybir.ActivationFunctionType.Tanh

    nc = bacc.Bacc("TRN2", target_bir_lowering=False, debug=False,
                   num_devices=N_CORES)

    hencT = nc.dram_tensor("hencT", [H, T], bf16, kind="ExternalInput").ap()
    hdecT = nc.dram_tensor("hdecT", [H, U], bf16, kind="ExternalInput").ap()
    w1T = nc.dram_tensor("w1T", [2 * H, HID], bf16, kind="ExternalInput").ap()
    w2T = nc.dram_tensor("w2T", [HID, V], bf16, kind="ExternalInput").ap()
    b1col = nc.dram_tensor("b1col", [HID, 1], f32, kind="ExternalInput").ap()
    b2rep = nc.dram_tensor("b2rep", [128, V], f32, kind="ExternalInput").ap()
    out = nc.dram_tensor("out", [TU, V], f32, kind="ExternalOutput").ap()

    def bcast3(ap2d, mid):
        """[P, N] AP -> [P, mid, N] with a stride-0 middle dim."""
        return bass.AP(tensor=ap2d.tensor, offset=ap2d.offset,
                       ap=[ap2d.ap[0], [0, mid], ap2d.ap[1]])

    def repeat3(ap2d, inner):
        """[P, N] AP -> [P, N, inner] with a stride-0 inner dim."""
        return bass.AP(tensor=ap2d.tensor, offset=ap2d.offset,
                       ap=[ap2d.ap[0], ap2d.ap[1], [0, inner]])

    with tile.TileContext(nc) as tc:
        with (
            tc.tile_pool(name="consts", bufs=1) as consts,
            tc.tile_pool(name="psum", bufs=1, space="PSUM") as psum,
            tc.tile_pool(name="prep", bufs=4) as prep,
            tc.tile_pool(name="hTp", bufs=3) as hTp,
            tc.tile_pool(name="outp", bufs=4) as outp,
        ):
            # ---- load inputs into SBUF ----
            henc_t = []
            for k in range(4):
                t_ = consts.tile([128, T], bf16, tag=f"hencT{k}", name=f"hencT{k}")
                nc.sync.dma_start(out=t_, in_=hencT[k * 128:(k + 1) * 128, :])
                henc_t.append(t_)
            hdec_t = []
            for k in range(4):
                t_ = consts.tile([128, U], bf16, tag=f"hdecT{k}", name=f"hdecT{k}")
                nc.sync.dma_start(out=t_, in_=hdecT[k * 128:(k + 1) * 128, :])
                hdec_t.append(t_)
            w1_t = []
            for k in range(8):
                t_ = consts.tile([128, HID], bf16, tag=f"w1T{k}", name=f"w1T{k}")
                nc.sync.dma_start(out=t_, in_=w1T[k * 128:(k + 1) * 128, :])
                w1_t.append(t_)
            w2_t = []
            for k in range(KK):
                t_ = consts.tile([128, V], bf16, tag=f"w2T{k}", name=f"w2T{k}")
                nc.gpsimd.dma_start(out=t_, in_=w2T[k * 128:(k + 1) * 128, :])
                w2_t.append(t_)
            b1_t = []
            for kk in range(KK):
                t_ = consts.tile([128, 1], f32, tag=f"b1{kk}", name=f"b1{kk}")
                nc.sync.dma_start(out=t_, in_=b1col[kk * 128:(kk + 1) * 128, :])
                b1_t.append(t_)
            b2_t = consts.tile([128, V], f32, tag="b2", name="b2")
            nc.gpsimd.dma_start(out=b2_t, in_=b2rep[:, :])

            # ---- GEMM1 (fp32): enc_pT [HID, T], dec_pT [HID, U] ----
            encbT = []
            for kk in range(KK):
                ps = psum.tile([128, T], f32, tag="g1", bufs=2, name=f"pse{kk}")
                for k in range(4):
                    nc.tensor.matmul(
                        ps,
                        lhsT=w1_t[k][:, kk * 128:(kk + 1) * 128],
                        rhs=henc_t[k],
                        start=(k == 0), stop=(k == 3),
                    )
                e_ = consts.tile([128, T], f32, tag=f"encbT{kk}", name=f"encbT{kk}")
                # encbT = enc_pT + b1 (per-partition bias)
                nc.scalar.add(out=e_, in_=ps, add=b1_t[kk])
                encbT.append(e_)
            decT = []
            for kk in range(KK):
                ps = psum.tile([128, U], f32, tag="g1", bufs=2, name=f"psd{kk}")
                for k in range(4):
                    nc.tensor.matmul(
                        ps,
                        lhsT=w1_t[4 + k][:, kk * 128:(kk + 1) * 128],
                        rhs=hdec_t[k],
                        start=(k == 0), stop=(k == 3),
                    )
                d_ = consts.tile([128, U], f32, tag=f"decT{kk}", name=f"decT{kk}")
                nc.scalar.copy(out=d_, in_=ps)
                decT.append(d_)

            # ---- main loop: build is emitted 2 chunks ahead of GEMM2 ----
            hT_by_chunk = {}

            def emit_build(c):
                hts = []
                for kk in range(KK):
                    pre = prep.tile([128, 1024], f32, tag=f"pre{kk}",
                                    name=f"pre{c}_{kk}", bufs=2)
                    pre_ap = pre[:, :]
                    out3 = bass.AP(tensor=pre_ap.tensor, offset=pre_ap.offset,
                                   ap=[pre_ap.ap[0], [64, 16], [1, 64]])
                    nc.vector.tensor_add(
                        out=out3,
                        in0=bcast3(decT[kk][:, :], 16),
                        in1=repeat3(encbT[kk][:, c * 16:(c + 1) * 16], 64),
                    )
                    ht = hTp.tile([128, 1024], bf16, tag=f"hT{kk}",
                                  name=f"hT{c}_{kk}", bufs=2)
                    nc.scalar.activation(out=ht, in_=pre, func=Tanh)
                    hts.append(ht)
                hT_by_chunk[c] = hts

            emit_build(0)
            emit_build(1)
            for c in range(N_CHUNKS):
                if c + 2 < N_CHUNKS:
                    emit_build(c + 2)
                hts = hT_by_chunk.pop(c)
                for mt in range(8):
                    ot = outp.tile([128, V], f32, tag="out", name=f"out{c}_{mt}")
                    ps2 = psum.tile([128, 1024], f32, tag="g2", bufs=3,
                                    name=f"ps2_{c}_{mt}")
                    for vc in range(2):
                        for kk in range(KK):
                            nc.tensor.matmul(
                                ps2[:, vc * 512:(vc + 1) * 512],
                                lhsT=hts[kk][:, mt * 128:(mt + 1) * 128],
                                rhs=w2_t[kk][:, vc * 512:(vc + 1) * 512],
                                start=(kk == 0), stop=(kk == KK - 1),
                            )
                    nc.vector.tensor_add(out=ot, in0=ps2, in1=b2_t)
                    r0 = c * 1024 + mt * 128
                    nc.sync.dma_start(out=out[r0:r0 + 128, :], in_=ot)

    nc.finalize()
    return nc


def _get_nc():
    if "nc" not in _CACHE:
        _CACHE["nc"] = _build_bass()
    return _CACHE["nc"]


def _make_in_maps(h_enc, h_dec, W1, b1, W2, b2):
    h_enc = np.asarray(h_enc, dtype=np.float32)
    h_dec = np.asarray(h_dec, dtype=np.float32)
    W1 = np.asarray(W1, dtype=np.float32)
    b1 = np.asarray(b1, dtype=np.float32)
    W2 = np.asarray(W2, dtype=np.float32)
    b2 = np.asarray(b2, dtype=np.float32)

    w1T = np.ascontiguousarray(W1.T)                    # [2H, HID] f32
    w2T = np.ascontiguousarray(W2.T).astype(BF16)       # [HID, V] bf16
    b1col = np.ascontiguousarray(b1.reshape(HID, 1))
    b2rep = np.ascontiguousarray(np.tile(b2.reshape(1, V), (128, 1)))

    in_maps = []
    for b in range(N_CORES):
        in_maps.append({
            "hencT": np.ascontiguousarray(h_enc[b].T).astype(BF16),  # [H, T]
            "hdecT": np.ascontiguousarray(h_dec[b].T).astype(BF16),  # [H, U]
            "w1T": w1T.astype(BF16),
            "w2T": w2T,
            "b1col": b1col,
            "b2rep": b2rep,
        })
    return in_maps


def _run(in_maps, **kwargs):
    from concourse import bass_utils
    nc = _get_nc()
    return bass_utils.run_bass_kernel_spmd(
        nc, in_maps, core_ids=list(range(N_CORES)), **kwargs)


def kernel(h_enc, h_dec, W1, b1, W2, b2):
    in_maps = _make_in_maps(h_enc, h_dec, W1, b1, W2, b2)
    res = _run(in_maps)
    outs = [r["out"].reshape(T, U, V) for r in res.results]
    return np.stack(outs, axis=0)
